# revision 14
# baseline (speedup 1.0000x reference)
"""Trainium2 Bass kernel for DynamicTaskMemoryInduction (capsule dynamic routing).

Math (reference semantics):
  Ws = W[0,:,0]  (W is a broadcast of shared weights over the in_caps axis C)
  hat_m[c,(n,d)] = m[c,:] @ Ws[(n,d),:]^T + b[0,n,c,d]      -> tm [C=64, N*D=768]
  hat_q[q,(n,d)] = q[q,:] @ Ws[(n,d),:]^T                   -> tq [Q, 768]  (c-independent)
  p = tanh(-pearson_d(tm, tq));  2x routing loop + final squash.

Because tq (and its routing updates) are c-independent, the per-(q,n,c,d)
tensors of the reference collapse to [Q,(n,d)] / [Q,(n,c)] shapes.

Key identities used on device (exact algebra, no approximation):
  - pearson numerator: num[q,n,c] = sum_d tmc[n,c,d] * tq[q,n,d]
      where tmc = tm - mean_d(tm) (centered constant), because sum_d tmc = 0.
  - recursive numerator: with u_i = lam_i * (tq_i - mean_d tq_i) (lam_i = 2^i),
      num'_{i+1} = num'_i + lam_i*(agree_i - mean_v_i * sm1)      (sm1 = sum_d tm)
      r_{i+1} = num' / sqrt(ssm * ssq(u) + lam^2 * EPS)
  - agree via the (constant) Gram matrix of tm:
      agree[q,n,c] = scale_v[q,n] * sum_{c'} coeff[q,n,c'] * G[n,c',c],
      G[n] = tm_n @ tm_n^T,  since v = scale_v * hat_v and hat_v = coeff @ tm_n.

Sharding: data-parallel over Q across 8 cores (64 queries/core, q on SBUF
partitions). Ws/m/b replicated; hat_m recomputed on every core (it rides along
in the same matmul as hat_q: lhsT = [qT | mT] is exactly 128 columns).
"""

import numpy as np

EPS = 1e-8
Q, I, C, N, D = 512, 768, 64, 4, 192
ND, NC = N * D, N * C
NCORES = 8
QL = Q // NCORES  # 64 queries per core

F32 = None  # set lazily (mybir import)


def _chunks_for_block(n):
    """(tile_k, p0, cnt) chunks covering nd rows [n*D, (n+1)*D) in 128-row tiles."""
    out = []
    lo, hi = n * D, (n + 1) * D
    k = lo // 128
    while lo < hi:
        k = lo // 128
        p0 = lo - k * 128
        cnt = min(hi - lo, 128 - p0)
        out.append((k, p0, cnt))
        lo += cnt
    return out


def build(reps=1, stop_at="full"):
    import concourse.bacc as bacc
    import concourse.tile as tile
    import concourse.mybir as mybir
    import concourse.masks as masks

    F32 = mybir.dt.float32
    AF = mybir.ActivationFunctionType
    OP = mybir.AluOpType
    AX = mybir.AxisListType

    nc = bacc.Bacc("TRN2", target_bir_lowering=False, debug=False,
                   num_devices=NCORES)

    wsT_d = nc.dram_tensor("wsT", [I, ND], F32, kind="ExternalInput").ap()
    qmT_d = nc.dram_tensor("qmT", [I, 128], F32, kind="ExternalInput").ap()
    b_d = nc.dram_tensor("b_r", [C, ND], F32, kind="ExternalInput").ap()
    out_d = nc.dram_tensor("out", [QL, ND], F32, kind="ExternalOutput").ap()

    KC = I // 128  # 6 contraction chunks

    with tile.TileContext(nc) as tc:
        with tc.tile_pool(name="const", bufs=1) as cp, \
             tc.tile_pool(name="sb768", bufs=2) as sp768, \
             tc.tile_pool(name="sb256", bufs=2) as sp256, \
             tc.tile_pool(name="sbsm", bufs=2) as spsm, \
             tc.tile_pool(name="sbt", bufs=1) as spt:

            ident = cp.tile([128, 128], F32, tag="ident")
            masks.make_identity(nc, ident[:])
            ones1 = cp.tile([1, 64], F32, tag="ones1")
            nc.gpsimd.memset(ones1[:], 1.0)
            eps_t = {}
            for lam2 in (1.0, 4.0, 16.0):
                t = cp.tile([QL, 1], F32, tag=f"eps{lam2}")
                nc.gpsimd.memset(t[:], lam2 * EPS)
                eps_t[lam2] = t

            for rep in range(reps):
                # ---------- load inputs ----------
                ws = []
                qm = []
                for k in range(KC):
                    w_k = cp.tile([128, ND], F32, tag=f"ws{k}")
                    nc.sync.dma_start(w_k[:], wsT_d[k * 128:(k + 1) * 128, :])
                    ws.append(w_k)
                    q_k = cp.tile([128, 128], F32, tag=f"qm{k}")
                    nc.sync.dma_start(q_k[:], qmT_d[k * 128:(k + 1) * 128, :])
                    qm.append(q_k)
                b_sb = cp.tile([C, ND], F32, tag="b")
                nc.sync.dma_start(b_sb[:], b_d[:])

                # ---------- phase A: [hat_q; hat_m] = qmT.T @ wsT ----------
                with tc.tile_pool(name="psA", bufs=1, space="PSUM") as psA:
                    ps_a = psA.tile([128, ND], F32, tag="a")
                    for k in range(KC):
                        nc.tensor.matmul(ps_a[:, 0:512], qm[k][:], ws[k][:, 0:512],
                                         start=(k == 0), stop=(k == KC - 1))
                    for k in range(KC):
                        nc.tensor.matmul(ps_a[:, 512:768], qm[k][:], ws[k][:, 512:768],
                                         start=(k == 0), stop=(k == KC - 1))

                    # tm = hat_m + b ; u0 = centered hat_q
                    tm = cp.tile([C, ND], F32, tag="tm")
                    nc.vector.tensor_add(tm[:], ps_a[64:128, :], b_sb[:])

                    s1q = spsm.tile([QL, N], F32, tag="s1q")
                    nc.vector.tensor_reduce(
                        out=s1q[:], in_=ps_a[0:64, :].rearrange("p (n d) -> p n d", n=N),
                        axis=AX.X, op=OP.add)
                    muq = spsm.tile([QL, N], F32, tag="muq")
                    nc.vector.tensor_scalar_mul(muq[:], s1q[:], 1.0 / D)
                    u = sp768.tile([QL, ND], F32, tag="u")
                    nc.vector.tensor_sub(
                        u[:].rearrange("p (n d) -> p n d", n=N),
                        ps_a[0:64, :].rearrange("p (n d) -> p n d", n=N),
                        muq[:].unsqueeze(2).broadcast_to([QL, N, D]))

                if stop_at == "phaseA":
                    nc.sync.dma_start(out_d[:], tm[:])
                    continue
                # ssq0 = sum_d u^2 per n
                squ = sp768.tile([QL, ND], F32, tag="squ")
                nc.vector.tensor_mul(squ[:], u[:], u[:])
                ssq = spsm.tile([QL, N], F32, tag="ssq")
                nc.vector.tensor_reduce(
                    out=ssq[:], in_=squ[:].rearrange("p (n d) -> p n d", n=N),
                    axis=AX.X, op=OP.add)

                # ---------- tm statistics / constants ----------
                s1m = spsm.tile([C, N], F32, tag="s1m")
                nc.vector.tensor_reduce(
                    out=s1m[:], in_=tm[:].rearrange("p (n d) -> p n d", n=N),
                    axis=AX.X, op=OP.add)
                mum = spsm.tile([C, N], F32, tag="mum")
                nc.vector.tensor_scalar_mul(mum[:], s1m[:], 1.0 / D)
                tmc = cp.tile([C, ND], F32, tag="tmc")  # centered tm (constant)
                nc.vector.tensor_sub(
                    tmc[:].rearrange("p (n d) -> p n d", n=N),
                    tm[:].rearrange("p (n d) -> p n d", n=N),
                    mum[:].unsqueeze(2).broadcast_to([C, N, D]))
                sqm = sp768.tile([C, ND], F32, tag="squ")
                nc.vector.tensor_mul(sqm[:], tmc[:], tmc[:])
                ssm = spsm.tile([C, N], F32, tag="ssm")
                nc.vector.tensor_reduce(
                    out=ssm[:], in_=sqm[:].rearrange("p (n d) -> p n d", n=N),
                    axis=AX.X, op=OP.add)

                if stop_at == "stats":
                    nc.sync.dma_start(out_d[:], tmc[:])
                    continue
                with tc.tile_pool(name="psS", bufs=2, space="PSUM") as psS, \
                     tc.tile_pool(name="psB", bufs=3, space="PSUM") as psB:
                    # ssm,s1m [64(c),4(n)] -> PE transpose -> [4,64] -> reshape
                    # DMA -> [1,(n,c)] -> gpsimd partition_broadcast -> [64,256]
                    ssm_b = cp.tile([QL, NC], F32, tag="ssm_b")
                    sm1_b = cp.tile([QL, NC], F32, tag="sm1_b")
                    for src, dst in ((ssm, ssm_b), (s1m, sm1_b)):
                        pt4 = psS.tile([N, C], F32, tag="tr4")
                        nc.tensor.transpose(pt4[:], src[:], ident[:64, :64])
                        t4 = spsm.tile([N, C], F32, tag="t4")
                        nc.vector.tensor_copy(t4[:], pt4[:])
                        row = spsm.tile([1, NC], F32, tag="row")
                        nc.sync.dma_start(
                            out=row[:].rearrange("x (n c) -> x n c", n=N),
                            in_=t4[:])
                        nc.gpsimd.partition_broadcast(dst[:], row[:])

                    if stop_at == "bcast":
                        nc.sync.dma_start(out_d[:, 0:NC], ssm_b[:])
                        nc.sync.dma_start(out_d[:, NC:2*NC], sm1_b[:])
                        continue
                    # tmT / u0T tiles: per n-block, split d into 128+64 so every
                    # matmul operand starts at partition 0 (base-partition-64
                    # operands fail at runtime on this HW path).
                    def transpose_blocks(src, pool, pfx):
                        parts = []
                        for n in range(N):
                            blk = []
                            for off, w in ((0, 128), (128, 64)):
                                pt = psS.tile([128, 64], F32, tag="tr")
                                nc.tensor.transpose(
                                    pt[:w, :],
                                    src[:, n * D + off:n * D + off + w],
                                    ident[:64, :64])
                                t_b = pool.tile([w, 64], F32, tag=f"{pfx}{n}_{off}")
                                nc.vector.tensor_copy(t_b[:], pt[:w, :])
                                blk.append((t_b, w))
                            parts.append(blk)
                        return parts

                    tmT = transpose_blocks(tm, cp, "tmT")
                    uT = transpose_blocks(u, spt, "uT")

                    if stop_at == "tmT":
                        for k in range(KC):
                            nc.sync.dma_start(out_d[0:64, k*64:(k+1)*64].rearrange("a b -> b a"), tmT[k][0:64, 0:64])
                        continue
                    # gram G[n] = tm_n @ tm_n^T  (on centered? NO: reference uses tm)
                    pg = psB.tile([C, NC], F32, tag="bc")
                    for n in range(N):
                        for j, (t_b, w) in enumerate(tmT[n]):
                            nc.tensor.matmul(pg[:, n * C:(n + 1) * C],
                                             t_b[:], t_b[:],
                                             start=(j == 0), stop=(j == 1))
                    gram = cp.tile([C, NC], F32, tag="gram")
                    nc.vector.tensor_copy(gram[:], pg[:])

                    # pear #1: num0[q,(n,c)] = sum_d u0T[d,q]*tmcT...  NOTE: use
                    # centered tm on the c side?  num = sum_d tmc[n,c,d]*u[q,n,d]
                    # we only built tmT (uncentered). sum_d u = 0 (u centered), so
                    # sum_d tm*u = sum_d tmc*u. Uncentered tmT works. ✓
                    if stop_at == "gram":
                        nc.sync.dma_start(out_d[:, 0:NC], gram[:])
                        continue
                    pp = psB.tile([QL, NC], F32, tag="bc")
                    for n in range(N):
                        for j, ((u_b, w), (t_b, _w)) in enumerate(zip(uT[n], tmT[n])):
                            nc.tensor.matmul(pp[:, n * C:(n + 1) * C],
                                             u_b[:], t_b[:],
                                             start=(j == 0), stop=(j == 1))
                    num = sp256.tile([QL, NC], F32, tag="num")
                    nc.vector.tensor_copy(num[:], pp[:])

                if stop_at == "setup":
                    nc.sync.dma_start(out_d[:], u[:])
                    continue

                def make_p(num_t, ssq_t, lam):
                    """p = tanh(-num / sqrt(ssm*ssq + lam^2*EPS)) ; [64,256]"""
                    den2 = sp256.tile([QL, NC], F32, tag="den2")
                    nc.vector.tensor_mul(
                        den2[:].rearrange("p (n c) -> p n c", n=N),
                        ssm_b[:].rearrange("p (n c) -> p n c", n=N),
                        ssq_t[:].unsqueeze(2).broadcast_to([QL, N, C]))
                    den = sp256.tile([QL, NC], F32, tag="den")
                    nc.scalar.activation(den[:], den2[:], AF.Sqrt,
                                         bias=eps_t[lam * lam][:], scale=1.0)
                    inv = sp256.tile([QL, NC], F32, tag="inv")
                    nc.vector.reciprocal(inv[:], den[:])
                    r_t = sp256.tile([QL, NC], F32, tag="r")
                    nc.vector.tensor_mul(r_t[:], num_t[:], inv[:])
                    p_t = sp256.tile([QL, NC], F32, tag="p")
                    nc.scalar.activation(p_t[:], r_t[:], AF.Tanh, bias=0.0, scale=-1.0)
                    return p_t

                def softmax_n(a_t):
                    """softmax over n of a [64,(n,c)] -> d_sm [64,256]"""
                    amax = spsm.tile([QL, C], F32, tag="amax")
                    nc.vector.tensor_reduce(
                        out=amax[:], in_=a_t[:].rearrange("p (n c) -> p c n", n=N),
                        axis=AX.X, op=OP.max)
                    e_t = sp256.tile([QL, NC], F32, tag="e")
                    nc.vector.tensor_sub(
                        e_t[:].rearrange("p (n c) -> p n c", n=N),
                        a_t[:].rearrange("p (n c) -> p n c", n=N),
                        amax[:].unsqueeze(1).broadcast_to([QL, N, C]))
                    nc.scalar.activation(e_t[:], e_t[:], AF.Exp, bias=0.0, scale=1.0)
                    rs = spsm.tile([QL, C], F32, tag="rs")
                    nc.vector.tensor_reduce(
                        out=rs[:], in_=e_t[:].rearrange("p (n c) -> p c n", n=N),
                        axis=AX.X, op=OP.add)
                    rsi = spsm.tile([QL, C], F32, tag="rsi")
                    nc.vector.reciprocal(rsi[:], rs[:])
                    d_sm = sp256.tile([QL, NC], F32, tag="dsm")
                    nc.vector.tensor_mul(
                        d_sm[:].rearrange("p (n c) -> p n c", n=N),
                        e_t[:].rearrange("p (n c) -> p n c", n=N),
                        rsi[:].unsqueeze(1).broadcast_to([QL, N, C]))
                    return d_sm

                p_t = make_p(num, ssq, 1.0)
                a_t = None

                with tc.tile_pool(name="psI", bufs=2, space="PSUM") as psI, \
                     tc.tile_pool(name="psH", bufs=2, space="PSUM") as psH:

                    def coeff_mm(coeff_t):
                        """transpose coeff blocks, matmul hv (and gram agree)."""
                        coeffT = sp256.tile([C, NC], F32, tag="coeffT")
                        for n in range(N):
                            pc = psI.tile([64, 64], F32, tag="ctr")
                            nc.tensor.transpose(pc[:], coeff_t[:, n * C:(n + 1) * C],
                                                ident[:64, :64])
                            nc.vector.tensor_copy(coeffT[:, n * C:(n + 1) * C], pc[:])
                        hv01 = psH.tile([QL, 2 * D], F32, tag="hv01")
                        hv23 = psH.tile([QL, 2 * D], F32, tag="hv23")
                        hv = [(hv01, 0), (hv01, 1), (hv23, 0), (hv23, 1)]
                        for n in range(N):
                            t, half = hv[n]
                            nc.tensor.matmul(t[:, half * D:(half + 1) * D],
                                             coeffT[:, n * C:(n + 1) * C],
                                             tm[:, n * D:(n + 1) * D],
                                             start=True, stop=True)
                        return coeffT, hv

                    def squash_stats(hv):
                        """returns scale_v [64,4], s1hv [64,4]"""
                        s1hv = spsm.tile([QL, N], F32, tag="s1hv")
                        sshv = spsm.tile([QL, N], F32, tag="sshv")
                        for n in range(N):
                            t, half = hv[n]
                            sl = t[:, half * D:(half + 1) * D]
                            nc.vector.tensor_reduce(out=s1hv[:, n:n + 1], in_=sl,
                                                    axis=AX.X, op=OP.add)
                            junk = sp768.tile([QL, D], F32, tag="junk")
                            nc.scalar.activation(junk[:], sl, AF.Square,
                                                 accum_out=sshv[:, n:n + 1])
                        t1 = spsm.tile([QL, N], F32, tag="t1")
                        nc.vector.tensor_scalar_add(t1[:], sshv[:], 1.0)
                        t1r = spsm.tile([QL, N], F32, tag="t1r")
                        nc.vector.reciprocal(t1r[:], t1[:])
                        t2 = spsm.tile([QL, N], F32, tag="t2")
                        nc.vector.tensor_mul(t2[:], sshv[:], t1r[:])
                        ds = spsm.tile([QL, N], F32, tag="ds")
                        nc.scalar.activation(ds[:], sshv[:], AF.Sqrt, bias=eps_t[1.0][:], scale=1.0)
                        dsr = spsm.tile([QL, N], F32, tag="dsr")
                        nc.vector.reciprocal(dsr[:], ds[:])
                        scale = spsm.tile([QL, N], F32, tag="scale")
                        nc.vector.tensor_mul(scale[:], t2[:], dsr[:])
                        return scale, s1hv

                    lam = 1.0
                    for it in (1, 2):
                        # coeff
                        coeff = sp256.tile([QL, NC], F32, tag="coeff")
                        if it == 1:
                            nc.vector.tensor_scalar_add(coeff[:], p_t[:], 1.0 / N)
                        else:
                            d_sm = softmax_n(a_t)
                            nc.vector.tensor_add(coeff[:], d_sm[:], p_t[:])

                        coeffT, hv = coeff_mm(coeff)
                        # agree (gram trick), raw (pre scale_v)
                        pag = psI.tile([QL, NC], F32, tag="ag")
                        for n in range(N):
                            nc.tensor.matmul(pag[:, n * C:(n + 1) * C],
                                             coeffT[:, n * C:(n + 1) * C],
                                             gram[:, n * C:(n + 1) * C],
                                             start=True, stop=True)

                        scale, s1hv = squash_stats(hv)

                        # agree = scale_v (bcast c) * pag
                        agree = sp256.tile([QL, NC], F32, tag="agree")
                        nc.vector.tensor_mul(
                            agree[:].rearrange("p (n c) -> p n c", n=N),
                            pag[:].rearrange("p (n c) -> p n c", n=N),
                            scale[:].unsqueeze(2).broadcast_to([QL, N, C]))

                        # a update: a += p * agree
                        pa = sp256.tile([QL, NC], F32, tag="pa")
                        nc.vector.tensor_mul(pa[:], p_t[:], agree[:])
                        if it == 1:
                            a_t = pa
                        else:
                            a_new = sp256.tile([QL, NC], F32, tag="a")
                            nc.vector.tensor_add(a_new[:], a_t[:], pa[:])
                            a_t = a_new

                        # mean_v = scale * s1hv / D
                        mv = spsm.tile([QL, N], F32, tag="mv")
                        nc.vector.tensor_mul(mv[:], scale[:], s1hv[:])
                        nc.vector.tensor_scalar_mul(mv[:], mv[:], 1.0 / D)

                        # num' += lam * (agree - mv*sm1)
                        q1 = sp256.tile([QL, NC], F32, tag="q1")
                        nc.vector.tensor_mul(
                            q1[:].rearrange("p (n c) -> p n c", n=N),
                            sm1_b[:].rearrange("p (n c) -> p n c", n=N),
                            mv[:].unsqueeze(2).broadcast_to([QL, N, C]))
                        q2 = sp256.tile([QL, NC], F32, tag="q2")
                        nc.vector.tensor_sub(q2[:], agree[:], q1[:])
                        num_new = sp256.tile([QL, NC], F32, tag="num")
                        nc.vector.scalar_tensor_tensor(
                            out=num_new[:], in0=q2[:], scalar=lam, in1=num[:],
                            op0=OP.mult, op1=OP.add)
                        num = num_new

                        # w1 = v - mv = hv*scale - mv (per n) ; u += lam*w1
                        w1 = sp768.tile([QL, ND], F32, tag="w1")
                        for n in range(N):
                            t, half = hv[n]
                            nc.vector.tensor_scalar(
                                out=w1[:, n * D:(n + 1) * D],
                                in0=t[:, half * D:(half + 1) * D],
                                scalar1=scale[:, n:n + 1], scalar2=mv[:, n:n + 1],
                                op0=OP.mult, op1=OP.subtract)
                        u_new = sp768.tile([QL, ND], F32, tag="u")
                        nc.vector.scalar_tensor_tensor(
                            out=u_new[:], in0=w1[:], scalar=lam, in1=u[:],
                            op0=OP.mult, op1=OP.add)
                        u = u_new
                        lam *= 2.0

                        squ2 = sp768.tile([QL, ND], F32, tag="squ")
                        nc.vector.tensor_mul(squ2[:], u[:], u[:])
                        ssq2 = spsm.tile([QL, N], F32, tag="ssq")
                        nc.vector.tensor_reduce(
                            out=ssq2[:], in_=squ2[:].rearrange("p (n d) -> p n d", n=N),
                            axis=AX.X, op=OP.add)
                        p_t = make_p(num, ssq2, lam)

                    # ---------- final: d=softmax(a), hv3, squash -> out ----------
                    d_sm = softmax_n(a_t)
                    coeff = sp256.tile([QL, NC], F32, tag="coeff")
                    nc.vector.tensor_add(coeff[:], d_sm[:], p_t[:])
                    coeffT, hv = coeff_mm(coeff)
                    scale, _s1 = squash_stats(hv)
                    out_sb = sp768.tile([QL, ND], F32, tag="out")
                    for n in range(N):
                        t, half = hv[n]
                        nc.vector.tensor_scalar_mul(
                            out_sb[:, n * D:(n + 1) * D],
                            t[:, half * D:(half + 1) * D], scale[:, n:n + 1])
                    nc.sync.dma_start(out_d[:], out_sb[:])

    nc.compile()
    return nc


_BUILD_CACHE = {}


def _get_built(reps=1):
    if reps not in _BUILD_CACHE:
        _BUILD_CACHE[reps] = build(reps)
    return _BUILD_CACHE[reps]


def _prep_inputs(m, q, W, b):
    """Host-side layout prep + per-core sharding."""
    m = np.asarray(m, dtype=np.float32)
    q = np.asarray(q, dtype=np.float32)
    W = np.asarray(W, dtype=np.float32)
    b = np.asarray(b, dtype=np.float32)
    Ws = W[0, :, 0, :, :].reshape(ND, I)          # [N*D, I]
    wsT = np.ascontiguousarray(Ws.T)              # [I, N*D]
    mT = m.T                                      # [I, C]
    b_r = np.ascontiguousarray(b[0].transpose(1, 0, 2).reshape(C, ND))
    in_maps = []
    for c in range(NCORES):
        qc = q[c * QL:(c + 1) * QL, :]            # [QL, I]
        qmT = np.ascontiguousarray(np.concatenate([qc.T, mT], axis=1))  # [I, 128]
        in_maps.append({"wsT": wsT, "qmT": qmT, "b_r": b_r})
    return in_maps


def kernel(m, q, W, b):
    from concourse.bass_utils import run_bass_kernel_spmd
    nc = _get_built(1)
    in_maps = _prep_inputs(m, q, W, b)
    res = run_bass_kernel_spmd(nc, in_maps, list(range(NCORES)))
    out = np.concatenate([res.results[c]["out"] for c in range(NCORES)], axis=0)
    return out.astype(np.float32)
